# revision 22
# baseline (speedup 1.0000x reference)
"""Trainium2 Bass kernel for the GODEFunc graph-ODE message-passing module.

Math (per batch b):
    xa   = sum_k conv_w[k] * (adj[k] @ x[b]) + conv_b
    W    = (w * clip(d,0,1)) @ w.T
    out  = tanh(0.5*sigmoid(alpha) * xa - 2*x[b] + x[b] @ W + x0[b]*sigmoid(beta))

Sharding: rows (nodes) split across 8 cores; each core computes its
1024-row slice of the output for all batches.  No collectives needed.

Host-side layout: adj is fed per-core TRANSPOSED, k-interleaved and
chunk-paired (adj2[G, p, j, k, r] = adj[k, row r, (2G+j)*128+p]) so the
contraction dim m lands on SBUF partitions and the PE consumes tiles
directly as lhsT — no on-device transposes, 16KB-contiguous DMA runs.
x/x0/x-rows/y are fed as [128, chunk, b, f]; alpha/beta as [128, nt].

Per-core kernel structure:
  - adj2 streams in as bf16 (cast during SWDGE DMA), 8MB per DMA
    covering both k planes of 8 contraction chunks; the last groups
    taper 4+2+2 so the final chunks stream without descriptor-ring
    drain gaps and the end-of-kernel tail is short.
  - No k-combine: x chunks are kept as TWO conv_w[k]-scaled bf16 copies
    and both k matmuls accumulate into the same PSUM region, keeping the
    buffer-recycle chain pure PE work so the DMA queue never stalls.
  - Main matmuls: psum_y[ntt] += adj_tile(k).T @ (conv_w[k] * x4[mc]).
  - x @ (W - 2I) in fp32 via PE transposes of x rows, interleaved into
    the inter-group PE gaps.
  - Last group is bank-ordered with per-bank epilogue + output DMA so
    tanh/writes overlap the final matmuls.
  - Epilogue: out = tanh(0.5*siga*psum_y + xw + x0*sigmoid(beta) + bias).
"""

import sys

for _p in ("/opt/trn_rl_repo",):
    if _p not in sys.path:
        sys.path.insert(0, _p)

from contextlib import ExitStack

import numpy as np

import concourse.bass as bass
import concourse.mybir as mybir
import concourse.tile as tile
from concourse import bacc
from concourse.bass_utils import run_bass_kernel_spmd
from concourse.masks import make_identity

dt = mybir.dt
AF = mybir.ActivationFunctionType
ALU = mybir.AluOpType

B, N, F, K = 4, 8192, 64, 2
N_CORES = 8
P = 128

# adj DMA groups (start_chunk, n_chunks); tapered tail (the last small
# groups fit in the SWDGE descriptor ring together, so they stream
# back-to-back with no drain gap before the final chunk)
GROUPS = [(0, 8), (8, 8), (16, 8), (24, 8), (32, 8), (40, 8), (48, 8),
          (56, 4), (60, 2), (62, 2)]


def build_kernel(n=N, n_cores=N_CORES, b=B, f=F, k_dim=K):
    """Build the per-core Bass module.  All cores run the same program on
    their own row shard."""
    ns = n // n_cores          # rows per core
    nt_cnt = ns // P           # output row tiles per core
    mc_cnt = n // P            # contraction chunks

    nc = bacc.Bacc(None, target_bir_lowering=False, debug=False)

    # chunk-paired layout: adj2[G, p, j, k, r] = adj[k, row r, (2G+j)*128+p]
    # -> 16KB contiguous per (G, p): longer HBM bursts under dual-NC load.
    adj2 = nc.dram_tensor("adj2", [mc_cnt // 2, P, 2, k_dim, ns], dt.float32,
                          kind="ExternalInput")
    x_t = nc.dram_tensor("x_t", [P, mc_cnt, b, f], dt.float32, kind="ExternalInput")
    xr_t = nc.dram_tensor("xr_t", [P, nt_cnt, b, f], dt.float32, kind="ExternalInput")
    x0_t = nc.dram_tensor("x0_t", [P, nt_cnt, b, f], dt.float32, kind="ExternalInput")
    alpha = nc.dram_tensor("alpha", [P, nt_cnt], dt.float32, kind="ExternalInput")
    beta = nc.dram_tensor("beta", [P, nt_cnt], dt.float32, kind="ExternalInput")
    w = nc.dram_tensor("w", [f, f], dt.float32, kind="ExternalInput")
    d = nc.dram_tensor("d", [f], dt.float32, kind="ExternalInput")
    conv_w = nc.dram_tensor("conv_w", [k_dim], dt.float32, kind="ExternalInput")
    conv_b = nc.dram_tensor("conv_b", [1], dt.float32, kind="ExternalInput")
    # output leaves the device as bf16 (tanh output is in [-1,1], so the
    # rounding is ~2e-3 absolute); host upcasts to f32
    y_t = nc.dram_tensor("y_t", [P, nt_cnt, b, f], dt.bfloat16,
                         kind="ExternalOutput")

    bf = b * f  # stacked batch-feature columns

    with tile.TileContext(nc) as tc, ExitStack() as ctx:
        const = ctx.enter_context(tc.tile_pool(name="const", bufs=1))
        adj_pool = ctx.enter_context(tc.tile_pool(name="adjp", bufs=3))
        adj_tail = ctx.enter_context(tc.tile_pool(name="adjt", bufs=1))
        xs_pool = ctx.enter_context(tc.tile_pool(name="xsp", bufs=3))
        xs_tail = ctx.enter_context(tc.tile_pool(name="xst", bufs=1))
        work = ctx.enter_context(tc.tile_pool(name="work", bufs=2))
        outp = ctx.enter_context(tc.tile_pool(name="outp", bufs=2))
        keep = ctx.enter_context(tc.tile_pool(name="keep", bufs=1))
        psy = ctx.enter_context(tc.tile_pool(name="psy", bufs=1, space="PSUM"))
        pst_pool = ctx.enter_context(tc.tile_pool(name="pst", bufs=2, space="PSUM"))
        paux = ctx.enter_context(tc.tile_pool(name="paux", bufs=2, space="PSUM"))

        def emit_adj_dma(c0, nch):
            ap = adj_pool if nch == 8 else adj_tail
            tsuf = str(nch) if nch == 8 else f"{nch}_{c0}"
            a_t = ap.tile([P, nch // 2, 2, k_dim, ns], dt.bfloat16,
                          tag=f"adj_{tsuf}", name="a_t")
            nc.gpsimd.dma_start(
                out=a_t[:],
                in_=adj2[c0 // 2 : (c0 + nch) // 2].rearrange(
                    "G p j k r -> p G j k r"
                ),
            )
            return a_t, tsuf

        def emit_xs_dma(c0, nch):
            xsp = xs_pool if nch == 8 else xs_tail
            tsuf = str(nch) if nch == 8 else f"{nch}_{c0}"
            xs0 = xsp.tile([P, nch, b, f], dt.bfloat16, tag=f"xs0_{tsuf}",
                           name="xs0")
            nc.gpsimd.dma_start(out=xs0[:], in_=x_t[:, c0 : c0 + nch])
            return xs0, xsp

        # Group 0's DMAs go first so nothing (not even the identity
        # builders, which also run on the gpsimd queue) delays the stream.
        g0_adj = emit_adj_dma(*GROUPS[0])
        g0_xs = emit_xs_dma(*GROUPS[0])
        # x chunks for the small tail groups are hoisted to the head so
        # the end-of-stream chain is adj-transfer -> matmuls only.
        tail_xs = {c0: emit_xs_dma(c0, nch) for c0, nch in GROUPS if nch != 8}

        # ---------------- constants / gates ----------------
        ident_f = const.tile([f, f], dt.float32, tag="ident_f")
        make_identity(nc, ident_f[:])
        ident_p = const.tile([P, P], dt.float32, tag="ident_p")
        make_identity(nc, ident_p[:])

        w_sb = const.tile([f, f], dt.float32, tag="w_sb")
        nc.sync.dma_start(out=w_sb[:], in_=w[:, :])
        d_sb = const.tile([f, 1], dt.float32, tag="d_sb")
        nc.sync.dma_start(out=d_sb[:], in_=d[:, None])
        cw_sb = const.tile([P, k_dim], dt.float32, tag="cw_sb")
        nc.sync.dma_start(out=cw_sb[:], in_=conv_w[None, :].to_broadcast((P, k_dim)))
        cb_sb = const.tile([P, 1], dt.float32, tag="cb_sb")
        nc.sync.dma_start(out=cb_sb[:], in_=conv_b[None, :].to_broadcast((P, 1)))

        al_sb = const.tile([P, nt_cnt], dt.float32, tag="al_sb")
        nc.sync.dma_start(out=al_sb[:], in_=alpha[:, :])
        be_sb = const.tile([P, nt_cnt], dt.float32, tag="be_sb")
        nc.sync.dma_start(out=be_sb[:], in_=beta[:, :])

        # x rows + x0 for this core, one DMA each
        xr_all = const.tile([P, nt_cnt, b, f], dt.float32, tag="xr_all")
        nc.sync.dma_start(out=xr_all[:], in_=xr_t[:, :])
        x0_all = const.tile([P, nt_cnt, b, f], dt.float32, tag="x0_all")
        nc.sync.dma_start(out=x0_all[:], in_=x0_t[:, :])

        # siga_half[p, nt] = 0.5 * sigmoid(alpha) — row scale for the adj term
        siga = const.tile([P, nt_cnt], dt.float32, tag="siga")
        nc.scalar.activation(siga[:], al_sb[:], AF.Sigmoid)
        siga_half = const.tile([P, nt_cnt], dt.float32, tag="siga_half")
        nc.vector.tensor_scalar(siga_half[:], siga[:], 0.5, None, ALU.mult)
        sigb = const.tile([P, nt_cnt], dt.float32, tag="sigb")
        nc.scalar.activation(sigb[:], be_sb[:], AF.Sigmoid)
        # bias_cb[p, nt] = 0.5 * sigmoid(alpha) * conv_b
        bias_cb = const.tile([P, nt_cnt], dt.float32, tag="bias_cb")
        nc.vector.tensor_scalar(
            bias_cb[:], siga_half[:], cb_sb[:, 0:1], None, ALU.mult
        )

        # ---------------- W' = (w * clip(d,0,1)) @ w.T - 2I ----------------
        pw = paux.tile([f, f], dt.float32, tag="paux")
        nc.tensor.matmul(
            pw[:], w_sb[:], ident_f[:], is_transpose=True, start=True, stop=True
        )
        wT = const.tile([f, f], dt.float32, tag="wT")
        nc.any.tensor_copy(wT[:], pw[:])
        dc = const.tile([f, 1], dt.float32, tag="dc")
        nc.vector.tensor_scalar(dc[:], d_sb[:], 0.0, 1.0, ALU.max, ALU.min)
        wdc = const.tile([f, f], dt.float32, tag="wdc")
        nc.vector.tensor_scalar(wdc[:], wT[:], dc[:], None, ALU.mult)
        pw2 = paux.tile([f, f], dt.float32, tag="paux")
        nc.tensor.matmul(pw2[:], wT[:], wdc[:], start=True, stop=True)
        wp = const.tile([f, f], dt.float32, tag="wp")
        nc.vector.scalar_tensor_tensor(
            wp[:], ident_f[:], -2.0, pw2[:], ALU.mult, ALU.add
        )

        # ---------------- psum accumulators: two row-tiles per bank ----------
        n_banks = (nt_cnt + 1) // 2
        psum_y = [
            psy.tile([P, 2 * bf], dt.float32, tag=f"y{i}", name=f"psum_y{i}")
            for i in range(n_banks)
        ]

        def y_region(ntt):
            return psum_y[ntt // 2][:, (ntt % 2) * bf : (ntt % 2 + 1) * bf]

        # out_final stages xwx0 in f32 (its values are O(5), too big for
        # bf16 staging); the tanh result lands in the bf16 out_bf buffer.
        out_final = keep.tile([P, nt_cnt, bf], dt.float32, tag="out_final")
        out_bf = keep.tile([P, nt_cnt, bf], dt.bfloat16, tag="out_bf")

        def emit_prologue(ntt):
            """xw = x_rows @ (W - 2I) for row-tile ntt; fp32 via PE transpose."""
            pxw = paux.tile([P, bf], dt.float32, tag="paux")
            for bb in range(b):
                pxT = pst_pool.tile([f, P], dt.float32, tag="pst")
                nc.tensor.matmul(
                    pxT[:], xr_all[:, ntt, bb, :], ident_p[:],
                    is_transpose=True, start=True, stop=True,
                )
                xT = work.tile([f, P], dt.float32, tag="xT")
                nc.any.tensor_copy(xT[:], pxT[:])
                nc.tensor.matmul(
                    pxw[:, bb * f : (bb + 1) * f], xT[:], wp[:],
                    start=True, stop=True,
                )
            # out_final[:, ntt] = x0 * sigmoid(beta) + xw
            nc.vector.scalar_tensor_tensor(
                out_final[:, ntt],
                x0_all[:, ntt].rearrange("p b f -> p (b f)"),
                sigb[:, ntt : ntt + 1],
                pxw[:],
                ALU.mult,
                ALU.add,
            )

        # first half of the prologue runs while the first adj group streams
        for ntt in range(4):
            emit_prologue(ntt)

        # ---------------- main loop: stream adj2, matmul both k --------------
        for gi, (c0, nch) in enumerate(GROUPS):
            if gi == 0:
                a_t, tsuf = g0_adj
                xs0, xsp = g0_xs
            else:
                a_t, tsuf = emit_adj_dma(c0, nch)
                if nch == 8:
                    xs0, xsp = emit_xs_dma(c0, nch)
                else:
                    xs0, xsp = tail_xs[c0]
            # xs0 is scaled in place by conv_w[0] after xs1 copies it
            # scaled by conv_w[1]; both feed the PSUM-side k-combine.
            xs1 = xsp.tile([P, nch, b, f], dt.bfloat16, tag=f"xs1_{tsuf}")
            nc.vector.tensor_scalar(
                xs1[:], xs0[:], cw_sb[:, 1:2], None, ALU.mult
            )
            nc.vector.tensor_scalar(
                xs0[:], xs0[:], cw_sb[:, 0:1], None, ALU.mult
            )
            x4s = [xs0, xs1]

            def emit_mm(g, kk, ntt):
                mc = c0 + g
                nc.tensor.matmul(
                    y_region(ntt),
                    a_t[:, g // 2, g % 2, kk, ntt * P : (ntt + 1) * P],
                    x4s[kk][:, g],
                    start=(mc == 0 and kk == 0),
                    stop=(mc == mc_cnt - 1 and kk == k_dim - 1),
                    skip_group_check=True,
                )

            def emit_epilogue(ntt):
                # out = tanh(0.5*siga*psum_y + xwx0 + bias)
                acc = outp.tile([P, bf], dt.float32, tag="eacc")
                nc.vector.scalar_tensor_tensor(
                    acc[:], y_region(ntt), siga_half[:, ntt : ntt + 1],
                    out_final[:, ntt], ALU.mult, ALU.add,
                )
                nc.scalar.activation(
                    out_bf[:, ntt], acc[:], AF.Tanh,
                    bias=bias_cb[:, ntt : ntt + 1],
                )

            if gi < len(GROUPS) - 1:
                for g in range(nch):
                    for kk in range(k_dim):
                        for ntt in range(nt_cnt):
                            emit_mm(g, kk, ntt)
                # remaining prologue tiles slot into the inter-group PE gaps
                if gi < 4:
                    emit_prologue(4 + gi)
            else:
                # last group: bank-ordered so epilogue + output writes
                # overlap the final matmuls
                for bank in range(n_banks):
                    for ntt in (2 * bank, 2 * bank + 1):
                        for g in range(nch):
                            for kk in range(k_dim):
                                emit_mm(g, kk, ntt)
                    emit_epilogue(2 * bank)
                    emit_epilogue(2 * bank + 1)
                    nc.sync.dma_start(
                        out=y_t[:, 2 * bank : 2 * bank + 2],
                        in_=out_bf[:, 2 * bank : 2 * bank + 2].rearrange(
                            "p t (b f) -> p t b f", b=b
                        ),
                    )

    nc.finalize()
    return nc


_NC_CACHE = {}


def _get_nc(key=(N, N_CORES, B, F, K)):
    if key not in _NC_CACHE:
        _NC_CACHE[key] = build_kernel(*key)
    return _NC_CACHE[key]


def make_in_maps(x, x0, adj, alpha, beta, w, d, conv_w, conv_b, n_cores=N_CORES):
    """Slice + re-lay the full inputs into per-core shards."""
    n = x.shape[1]
    ns = n // n_cores
    b, f = x.shape[0], x.shape[2]
    nt = ns // P
    mc = n // P
    f32 = np.float32

    # x_t[p, mc, b, f] = x[b, mc*128+p, f] — shared by all cores
    x_t = np.ascontiguousarray(
        x.reshape(b, mc, P, f).transpose(2, 1, 0, 3), dtype=f32
    )

    in_maps = []
    for c in range(n_cores):
        rows = slice(c * ns, (c + 1) * ns)
        # adj2[G, p, j, k, r] = adj[k, c*ns + r, (2G+j)*128 + p]
        kd = adj.shape[0]
        adj2c = np.ascontiguousarray(
            adj[:, rows, :]
            .transpose(2, 0, 1)
            .reshape(mc // 2, 2, P, kd, ns)
            .transpose(0, 2, 1, 3, 4),
            dtype=f32,
        )
        x0_tc = np.ascontiguousarray(
            x0[:, rows, :].reshape(b, nt, P, f).transpose(2, 1, 0, 3), dtype=f32
        )
        xr_tc = np.ascontiguousarray(x_t[:, c * nt : (c + 1) * nt], dtype=f32)
        in_maps.append(
            {
                "adj2": adj2c,
                "x_t": x_t,
                "xr_t": xr_tc,
                "x0_t": x0_tc,
                "alpha": np.ascontiguousarray(
                    alpha[rows].reshape(nt, P).T, dtype=f32
                ),
                "beta": np.ascontiguousarray(
                    beta[rows].reshape(nt, P).T, dtype=f32
                ),
                "w": np.ascontiguousarray(w, dtype=f32),
                "d": np.ascontiguousarray(d, dtype=f32),
                "conv_w": np.ascontiguousarray(conv_w, dtype=f32),
                "conv_b": np.ascontiguousarray(conv_b, dtype=f32),
            }
        )
    return in_maps


def kernel(x, x0, adj, alpha, beta, w, d, conv_w, conv_b):
    x = np.asarray(x)
    x0 = np.asarray(x0)
    adj = np.asarray(adj)
    alpha = np.asarray(alpha)
    beta = np.asarray(beta)
    w = np.asarray(w)
    d = np.asarray(d)
    conv_w = np.asarray(conv_w)
    conv_b = np.asarray(conv_b)

    b, n, f = x.shape
    ns = n // N_CORES

    nc = _get_nc()
    in_maps = make_in_maps(x, x0, adj, alpha, beta, w, d, conv_w, conv_b)
    res = run_bass_kernel_spmd(nc, in_maps, core_ids=list(range(N_CORES)))
    # y_t[p, nt, b, f] -> y[b, c*ns + nt*128 + p, f]
    parts = [
        res.results[c]["y_t"].transpose(2, 1, 0, 3).reshape(b, ns, f)
        for c in range(N_CORES)
    ]
    out = np.concatenate(parts, axis=1)
    return out.astype(np.float32)


# revision 23
# speedup vs baseline: 1.0088x; 1.0088x over previous
"""Trainium2 Bass kernel for the GODEFunc graph-ODE message-passing module.

Math (per batch b):
    xa   = sum_k conv_w[k] * (adj[k] @ x[b]) + conv_b
    W    = (w * clip(d,0,1)) @ w.T
    out  = tanh(0.5*sigmoid(alpha) * xa - 2*x[b] + x[b] @ W + x0[b]*sigmoid(beta))

Sharding: rows (nodes) split across 8 cores; each core computes its
1024-row slice of the output for all batches.  No collectives needed.

Host-side layout: adj is fed per-core TRANSPOSED, k-interleaved and
chunk-paired (adj2[G, p, j, k, r] = adj[k, row r, (2G+j)*128+p]) so the
contraction dim m lands on SBUF partitions and the PE consumes tiles
directly as lhsT — no on-device transposes, 16KB-contiguous DMA runs.
x/x0/x-rows/y are fed as [128, chunk, b, f]; alpha/beta as [128, nt].

Per-core kernel structure:
  - adj2 streams in as bf16 (cast during SWDGE DMA), 8MB per DMA
    covering both k planes of 8 contraction chunks; the last groups
    taper 4+2+2 so the final chunks stream without descriptor-ring
    drain gaps and the end-of-kernel tail is short.
  - No k-combine: x chunks are kept as TWO conv_w[k]-scaled bf16 copies
    and both k matmuls accumulate into the same PSUM region, keeping the
    buffer-recycle chain pure PE work so the DMA queue never stalls.
  - Main matmuls: psum_y[ntt] += adj_tile(k).T @ (conv_w[k] * x4[mc]).
  - x @ (W - 2I) in fp32 via PE transposes of x rows, interleaved into
    the inter-group PE gaps.
  - Last group is bank-ordered with per-bank epilogue + output DMA so
    tanh/writes overlap the final matmuls.
  - Epilogue: out = tanh(0.5*siga*psum_y + xw + x0*sigmoid(beta) + bias).
"""

import sys

for _p in ("/opt/trn_rl_repo",):
    if _p not in sys.path:
        sys.path.insert(0, _p)

from contextlib import ExitStack

import numpy as np

import concourse.bass as bass
import concourse.mybir as mybir
import concourse.tile as tile
from concourse import bacc
from concourse.bass_utils import run_bass_kernel_spmd
from concourse.masks import make_identity

dt = mybir.dt
AF = mybir.ActivationFunctionType
ALU = mybir.AluOpType

B, N, F, K = 4, 8192, 64, 2
N_CORES = 8
P = 128

# adj DMA groups (start_chunk, n_chunks); tapered tail (the last small
# groups fit in the SWDGE descriptor ring together, so they stream
# back-to-back with no drain gap before the final chunk)
GROUPS = [(0, 4), (4, 4), (8, 8), (16, 8), (24, 8), (32, 8), (40, 8),
          (48, 8), (56, 4), (60, 2), (62, 2)]


def build_kernel(n=N, n_cores=N_CORES, b=B, f=F, k_dim=K):
    """Build the per-core Bass module.  All cores run the same program on
    their own row shard."""
    ns = n // n_cores          # rows per core
    nt_cnt = ns // P           # output row tiles per core
    mc_cnt = n // P            # contraction chunks

    nc = bacc.Bacc(None, target_bir_lowering=False, debug=False)

    # chunk-paired layout: adj2[G, p, j, k, r] = adj[k, row r, (2G+j)*128+p]
    # -> 16KB contiguous per (G, p): longer HBM bursts under dual-NC load.
    adj2 = nc.dram_tensor("adj2", [mc_cnt // 2, P, 2, k_dim, ns], dt.float32,
                          kind="ExternalInput")
    x_t = nc.dram_tensor("x_t", [P, mc_cnt, b, f], dt.float32, kind="ExternalInput")
    xr_t = nc.dram_tensor("xr_t", [P, nt_cnt, b, f], dt.float32, kind="ExternalInput")
    x0_t = nc.dram_tensor("x0_t", [P, nt_cnt, b, f], dt.float32, kind="ExternalInput")
    alpha = nc.dram_tensor("alpha", [P, nt_cnt], dt.float32, kind="ExternalInput")
    beta = nc.dram_tensor("beta", [P, nt_cnt], dt.float32, kind="ExternalInput")
    w = nc.dram_tensor("w", [f, f], dt.float32, kind="ExternalInput")
    d = nc.dram_tensor("d", [f], dt.float32, kind="ExternalInput")
    conv_w = nc.dram_tensor("conv_w", [k_dim], dt.float32, kind="ExternalInput")
    conv_b = nc.dram_tensor("conv_b", [1], dt.float32, kind="ExternalInput")
    # output leaves the device as bf16 (tanh output is in [-1,1], so the
    # rounding is ~2e-3 absolute); host upcasts to f32
    y_t = nc.dram_tensor("y_t", [P, nt_cnt, b, f], dt.bfloat16,
                         kind="ExternalOutput")

    bf = b * f  # stacked batch-feature columns

    with tile.TileContext(nc) as tc, ExitStack() as ctx:
        const = ctx.enter_context(tc.tile_pool(name="const", bufs=1))
        adj_pool = ctx.enter_context(tc.tile_pool(name="adjp", bufs=3))
        adj_tail = ctx.enter_context(tc.tile_pool(name="adjt", bufs=1))
        xs_pool = ctx.enter_context(tc.tile_pool(name="xsp", bufs=3))
        xs_tail = ctx.enter_context(tc.tile_pool(name="xst", bufs=1))
        work = ctx.enter_context(tc.tile_pool(name="work", bufs=2))
        outp = ctx.enter_context(tc.tile_pool(name="outp", bufs=2))
        keep = ctx.enter_context(tc.tile_pool(name="keep", bufs=1))
        psy = ctx.enter_context(tc.tile_pool(name="psy", bufs=1, space="PSUM"))
        pst_pool = ctx.enter_context(tc.tile_pool(name="pst", bufs=2, space="PSUM"))
        paux = ctx.enter_context(tc.tile_pool(name="paux", bufs=2, space="PSUM"))

        def emit_adj_dma(c0, nch):
            tail = c0 >= 56
            ap = adj_tail if tail else adj_pool
            tsuf = f"{nch}_{c0}" if tail else "8"
            a_t = ap.tile([P, nch // 2, 2, k_dim, ns], dt.bfloat16,
                          tag=f"adj_{tsuf}", name="a_t")
            nc.gpsimd.dma_start(
                out=a_t[:],
                in_=adj2[c0 // 2 : (c0 + nch) // 2].rearrange(
                    "G p j k r -> p G j k r"
                ),
            )
            return a_t, tsuf

        def emit_xs_dma(c0, nch):
            tail = c0 >= 56
            xsp = xs_tail if tail else xs_pool
            tsuf = f"{nch}_{c0}" if tail else "8"
            xs0 = xsp.tile([P, nch, b, f], dt.bfloat16, tag=f"xs0_{tsuf}",
                           name="xs0")
            nc.gpsimd.dma_start(out=xs0[:], in_=x_t[:, c0 : c0 + nch])
            return xs0, xsp

        # Group 0's DMAs go first so nothing (not even the identity
        # builders, which also run on the gpsimd queue) delays the stream.
        g0_adj = emit_adj_dma(*GROUPS[0])
        g0_xs = emit_xs_dma(*GROUPS[0])
        # x chunks for the small tail groups are hoisted to the head so
        # the end-of-stream chain is adj-transfer -> matmuls only.
        tail_xs = {c0: emit_xs_dma(c0, nch) for c0, nch in GROUPS if c0 >= 56}

        # ---------------- constants / gates ----------------
        ident_f = const.tile([f, f], dt.float32, tag="ident_f")
        make_identity(nc, ident_f[:])
        ident_p = const.tile([P, P], dt.float32, tag="ident_p")
        make_identity(nc, ident_p[:])

        w_sb = const.tile([f, f], dt.float32, tag="w_sb")
        nc.sync.dma_start(out=w_sb[:], in_=w[:, :])
        d_sb = const.tile([f, 1], dt.float32, tag="d_sb")
        nc.sync.dma_start(out=d_sb[:], in_=d[:, None])
        cw_sb = const.tile([P, k_dim], dt.float32, tag="cw_sb")
        nc.sync.dma_start(out=cw_sb[:], in_=conv_w[None, :].to_broadcast((P, k_dim)))
        cb_sb = const.tile([P, 1], dt.float32, tag="cb_sb")
        nc.sync.dma_start(out=cb_sb[:], in_=conv_b[None, :].to_broadcast((P, 1)))

        al_sb = const.tile([P, nt_cnt], dt.float32, tag="al_sb")
        nc.sync.dma_start(out=al_sb[:], in_=alpha[:, :])
        be_sb = const.tile([P, nt_cnt], dt.float32, tag="be_sb")
        nc.sync.dma_start(out=be_sb[:], in_=beta[:, :])

        # x rows + x0 for this core, one DMA each
        xr_all = const.tile([P, nt_cnt, b, f], dt.float32, tag="xr_all")
        nc.sync.dma_start(out=xr_all[:], in_=xr_t[:, :])
        x0_all = const.tile([P, nt_cnt, b, f], dt.float32, tag="x0_all")
        nc.sync.dma_start(out=x0_all[:], in_=x0_t[:, :])

        # siga_half[p, nt] = 0.5 * sigmoid(alpha) — row scale for the adj term
        siga = const.tile([P, nt_cnt], dt.float32, tag="siga")
        nc.scalar.activation(siga[:], al_sb[:], AF.Sigmoid)
        siga_half = const.tile([P, nt_cnt], dt.float32, tag="siga_half")
        nc.vector.tensor_scalar(siga_half[:], siga[:], 0.5, None, ALU.mult)
        sigb = const.tile([P, nt_cnt], dt.float32, tag="sigb")
        nc.scalar.activation(sigb[:], be_sb[:], AF.Sigmoid)
        # bias_cb[p, nt] = 0.5 * sigmoid(alpha) * conv_b
        bias_cb = const.tile([P, nt_cnt], dt.float32, tag="bias_cb")
        nc.vector.tensor_scalar(
            bias_cb[:], siga_half[:], cb_sb[:, 0:1], None, ALU.mult
        )

        # ---------------- W' = (w * clip(d,0,1)) @ w.T - 2I ----------------
        pw = paux.tile([f, f], dt.float32, tag="paux")
        nc.tensor.matmul(
            pw[:], w_sb[:], ident_f[:], is_transpose=True, start=True, stop=True
        )
        wT = const.tile([f, f], dt.float32, tag="wT")
        nc.any.tensor_copy(wT[:], pw[:])
        dc = const.tile([f, 1], dt.float32, tag="dc")
        nc.vector.tensor_scalar(dc[:], d_sb[:], 0.0, 1.0, ALU.max, ALU.min)
        wdc = const.tile([f, f], dt.float32, tag="wdc")
        nc.vector.tensor_scalar(wdc[:], wT[:], dc[:], None, ALU.mult)
        pw2 = paux.tile([f, f], dt.float32, tag="paux")
        nc.tensor.matmul(pw2[:], wT[:], wdc[:], start=True, stop=True)
        wp = const.tile([f, f], dt.float32, tag="wp")
        nc.vector.scalar_tensor_tensor(
            wp[:], ident_f[:], -2.0, pw2[:], ALU.mult, ALU.add
        )

        # ---------------- psum accumulators: two row-tiles per bank ----------
        n_banks = (nt_cnt + 1) // 2
        psum_y = [
            psy.tile([P, 2 * bf], dt.float32, tag=f"y{i}", name=f"psum_y{i}")
            for i in range(n_banks)
        ]

        def y_region(ntt):
            return psum_y[ntt // 2][:, (ntt % 2) * bf : (ntt % 2 + 1) * bf]

        # out_final stages xwx0 in f32 (its values are O(5), too big for
        # bf16 staging); the tanh result lands in the bf16 out_bf buffer.
        out_final = keep.tile([P, nt_cnt, bf], dt.float32, tag="out_final")
        out_bf = keep.tile([P, nt_cnt, bf], dt.bfloat16, tag="out_bf")

        def emit_prologue(ntt):
            """xw = x_rows @ (W - 2I) for row-tile ntt; fp32 via PE transpose."""
            pxw = paux.tile([P, bf], dt.float32, tag="paux")
            for bb in range(b):
                pxT = pst_pool.tile([f, P], dt.float32, tag="pst")
                nc.tensor.matmul(
                    pxT[:], xr_all[:, ntt, bb, :], ident_p[:],
                    is_transpose=True, start=True, stop=True,
                )
                xT = work.tile([f, P], dt.float32, tag="xT")
                nc.any.tensor_copy(xT[:], pxT[:])
                nc.tensor.matmul(
                    pxw[:, bb * f : (bb + 1) * f], xT[:], wp[:],
                    start=True, stop=True,
                )
            # out_final[:, ntt] = x0 * sigmoid(beta) + xw
            nc.vector.scalar_tensor_tensor(
                out_final[:, ntt],
                x0_all[:, ntt].rearrange("p b f -> p (b f)"),
                sigb[:, ntt : ntt + 1],
                pxw[:],
                ALU.mult,
                ALU.add,
            )

        # first half of the prologue runs while the first adj group streams
        for ntt in range(4):
            emit_prologue(ntt)

        # ---------------- main loop: stream adj2, matmul both k --------------
        for gi, (c0, nch) in enumerate(GROUPS):
            if gi == 0:
                a_t, tsuf = g0_adj
                xs0, xsp = g0_xs
            else:
                a_t, tsuf = emit_adj_dma(c0, nch)
                if c0 >= 56:
                    xs0, xsp = tail_xs[c0]
                else:
                    xs0, xsp = emit_xs_dma(c0, nch)
            # xs0 is scaled in place by conv_w[0] after xs1 copies it
            # scaled by conv_w[1]; both feed the PSUM-side k-combine.
            xs1 = xsp.tile([P, nch, b, f], dt.bfloat16, tag=f"xs1_{tsuf}")
            nc.vector.tensor_scalar(
                xs1[:], xs0[:], cw_sb[:, 1:2], None, ALU.mult
            )
            nc.vector.tensor_scalar(
                xs0[:], xs0[:], cw_sb[:, 0:1], None, ALU.mult
            )
            x4s = [xs0, xs1]

            def emit_mm(g, kk, ntt):
                mc = c0 + g
                nc.tensor.matmul(
                    y_region(ntt),
                    a_t[:, g // 2, g % 2, kk, ntt * P : (ntt + 1) * P],
                    x4s[kk][:, g],
                    start=(mc == 0 and kk == 0),
                    stop=(mc == mc_cnt - 1 and kk == k_dim - 1),
                    skip_group_check=True,
                )

            def emit_epilogue(ntt):
                # out = tanh(0.5*siga*psum_y + xwx0 + bias)
                acc = outp.tile([P, bf], dt.float32, tag="eacc")
                nc.vector.scalar_tensor_tensor(
                    acc[:], y_region(ntt), siga_half[:, ntt : ntt + 1],
                    out_final[:, ntt], ALU.mult, ALU.add,
                )
                nc.scalar.activation(
                    out_bf[:, ntt], acc[:], AF.Tanh,
                    bias=bias_cb[:, ntt : ntt + 1],
                )

            if gi < len(GROUPS) - 1:
                for g in range(nch):
                    for kk in range(k_dim):
                        for ntt in range(nt_cnt):
                            emit_mm(g, kk, ntt)
                # remaining prologue tiles slot into the inter-group PE gaps
                if gi < 4:
                    emit_prologue(4 + gi)
            else:
                # last group: bank-ordered so epilogue + output writes
                # overlap the final matmuls
                for bank in range(n_banks):
                    for ntt in (2 * bank, 2 * bank + 1):
                        for g in range(nch):
                            for kk in range(k_dim):
                                emit_mm(g, kk, ntt)
                    emit_epilogue(2 * bank)
                    emit_epilogue(2 * bank + 1)
                    nc.sync.dma_start(
                        out=y_t[:, 2 * bank : 2 * bank + 2],
                        in_=out_bf[:, 2 * bank : 2 * bank + 2].rearrange(
                            "p t (b f) -> p t b f", b=b
                        ),
                    )

    nc.finalize()
    return nc


_NC_CACHE = {}


def _get_nc(key=(N, N_CORES, B, F, K)):
    if key not in _NC_CACHE:
        _NC_CACHE[key] = build_kernel(*key)
    return _NC_CACHE[key]


def make_in_maps(x, x0, adj, alpha, beta, w, d, conv_w, conv_b, n_cores=N_CORES):
    """Slice + re-lay the full inputs into per-core shards."""
    n = x.shape[1]
    ns = n // n_cores
    b, f = x.shape[0], x.shape[2]
    nt = ns // P
    mc = n // P
    f32 = np.float32

    # x_t[p, mc, b, f] = x[b, mc*128+p, f] — shared by all cores
    x_t = np.ascontiguousarray(
        x.reshape(b, mc, P, f).transpose(2, 1, 0, 3), dtype=f32
    )

    in_maps = []
    for c in range(n_cores):
        rows = slice(c * ns, (c + 1) * ns)
        # adj2[G, p, j, k, r] = adj[k, c*ns + r, (2G+j)*128 + p]
        kd = adj.shape[0]
        adj2c = np.ascontiguousarray(
            adj[:, rows, :]
            .transpose(2, 0, 1)
            .reshape(mc // 2, 2, P, kd, ns)
            .transpose(0, 2, 1, 3, 4),
            dtype=f32,
        )
        x0_tc = np.ascontiguousarray(
            x0[:, rows, :].reshape(b, nt, P, f).transpose(2, 1, 0, 3), dtype=f32
        )
        xr_tc = np.ascontiguousarray(x_t[:, c * nt : (c + 1) * nt], dtype=f32)
        in_maps.append(
            {
                "adj2": adj2c,
                "x_t": x_t,
                "xr_t": xr_tc,
                "x0_t": x0_tc,
                "alpha": np.ascontiguousarray(
                    alpha[rows].reshape(nt, P).T, dtype=f32
                ),
                "beta": np.ascontiguousarray(
                    beta[rows].reshape(nt, P).T, dtype=f32
                ),
                "w": np.ascontiguousarray(w, dtype=f32),
                "d": np.ascontiguousarray(d, dtype=f32),
                "conv_w": np.ascontiguousarray(conv_w, dtype=f32),
                "conv_b": np.ascontiguousarray(conv_b, dtype=f32),
            }
        )
    return in_maps


def kernel(x, x0, adj, alpha, beta, w, d, conv_w, conv_b):
    x = np.asarray(x)
    x0 = np.asarray(x0)
    adj = np.asarray(adj)
    alpha = np.asarray(alpha)
    beta = np.asarray(beta)
    w = np.asarray(w)
    d = np.asarray(d)
    conv_w = np.asarray(conv_w)
    conv_b = np.asarray(conv_b)

    b, n, f = x.shape
    ns = n // N_CORES

    nc = _get_nc()
    in_maps = make_in_maps(x, x0, adj, alpha, beta, w, d, conv_w, conv_b)
    res = run_bass_kernel_spmd(nc, in_maps, core_ids=list(range(N_CORES)))
    # y_t[p, nt, b, f] -> y[b, c*ns + nt*128 + p, f]
    parts = [
        res.results[c]["y_t"].transpose(2, 1, 0, 3).reshape(b, ns, f)
        for c in range(N_CORES)
    ]
    out = np.concatenate(parts, axis=1)
    return out.astype(np.float32)
